# revision 7
# baseline (speedup 1.0000x reference)
"""Holt-Winters exponential smoothing (level/trend/seasonal, P=7) on 8 Trainium2
NeuronCores.

Math: the per-row recurrence is linear in a 9-dim state
s = [level, trend, buf_0..buf_6]:  s_t = A_{t%7} s_{t-1} + c_{t%7} x_t.
Steps t=1..4095 are processed in 117 chunks of C=35 steps (35 % 7 == 0 so every
chunk sees the same slot pattern and shares one coefficient set), grouped into
9 groups of G=13 chunks.  Per chunk the outputs are a matmul
  Y_c (105,B) = [Wm | U]^T @ [X_c (35,B); sigma_c (9,B)]
and the chunk-entry states sigma_c come from a per-group prefix-scan matmul
over the group's stacked inputs.  All heavy compute runs on the TensorEngine;
the only sequential dependency is the 9-link group chain.

Sharding: pure data-parallel over the batch axis (1024 rows per core).
"""

import numpy as np

P = 7
C = 35            # chunk size (steps); 35 % 7 == 0
G = 13            # chunks per group
NG = 9            # groups; NG*G*C == L-1
L = 4096
B = 8192
NCORES = 8
BL = B // NCORES  # 1024 batch rows per core
NHALF = 512       # matmul moving-dim tile (fp32 PSUM bank limit)

DT_MM = "float32"  # matmul operand dtype: "float32" (exact) or "float32r" (fast)


def _sigmoid(z):
    return 1.0 / (1.0 + np.exp(-z))


def _step_mats(a, b, g):
    """A_i (9x9), c_i (9,) for seasonal slot i, float64."""
    A, c = [], []
    for i in range(P):
        col = 2 + i
        Ai = np.zeros((9, 9), np.float64)
        ci = np.zeros(9, np.float64)
        Ai[0, 0] = 1 - a
        Ai[0, 1] = 1 - a
        Ai[0, col] += -a
        Ai[1, 0] = -a * b
        Ai[1, 1] = 1 - a * b
        Ai[1, col] += -a * b
        for j in range(P):
            Ai[2 + j, 2 + j] = 1.0
        Ai[col, :] = 0.0
        Ai[col, 0] = -g * (1 - a)
        Ai[col, 1] = -g * (1 - a)
        Ai[col, col] = g * a + 1 - g
        ci[0] = a
        ci[1] = a * b
        ci[col] = g * (1 - a)
        A.append(Ai)
        c.append(ci)
    return A, c


def _build_coeffs(alpha, beta, gamma):
    """Host-precomputed stationary matrices (float64 -> float32).

    Returns dict of lhsT-layout arrays:
      wmain  (44, 105): pass-2; K rows 0..34 = X coeffs, 35..43 = sigma coeffs
      wscan1 (126, 126): scan mm over previous group's state tile (rows 117..125)
      wqv    (13, 35, 126): scan mm K-tile for each chunk position in the group
      winit  (7, 126): init mm: out rows 0..2 = y_0, rows 117..125 = s_0
    """
    a, b, g = _sigmoid(alpha), _sigmoid(beta), _sigmoid(gamma)
    A, c = _step_mats(a, b, g)
    slots = [(1 + k) % P for k in range(C)]

    Phi = np.zeros((C, 9, 9), np.float64)
    w = np.zeros((C, C, 9), np.float64)
    cur = np.eye(9)
    for k in range(C):
        i = slots[k]
        if k > 0:
            w[k, :k] = w[k - 1, :k] @ A[i].T
        w[k, k] = c[i]
        cur = A[i] @ cur
        Phi[k] = cur
    T = Phi[C - 1]
    V = w[C - 1].T.copy()  # (9, C)

    wmain = np.zeros((44, 105), np.float64)
    for k in range(C):
        sel = [0, 1, 2 + slots[k]]
        wmain[35:44, 3 * k:3 * k + 3] = Phi[k][sel].T          # U part
        for j in range(k + 1):
            wmain[j, 3 * k:3 * k + 3] = w[k, j][sel]            # Wm part

    Tpow = [np.eye(9)]
    for _ in range(G + 1):
        Tpow.append(T @ Tpow[-1])

    wscan1 = np.zeros((126, 126), np.float64)
    for j in range(G + 1):
        wscan1[117:126, 9 * j:9 * j + 9] = Tpow[j].T
    wqv = np.zeros((G, C, 126), np.float64)
    for i in range(G):
        for j in range(i + 1, G + 1):
            wqv[i, :, 9 * j:9 * j + 9] = (Tpow[j - 1 - i] @ V).T

    winit = np.zeros((7, 126), np.float64)
    winit[0, 0] = 1.0
    winit[0, 1] = -1.0
    winit[1, 1] = 1.0
    winit[0, 117] = 1.0
    winit[0, 118] = -1.0
    winit[1, 118] = 1.0
    for j in range(P):
        winit[j, 119 + j] += 1.0
        winit[0, 119 + j] += -1.0

    return {k: v.astype(np.float32) for k, v in
            dict(wmain=wmain, wscan1=wscan1, wqv=wqv, winit=winit).items()}


def build_bass(bl=BL, dt_mm=DT_MM):
    """Build the per-core Bass module (SPMD: same module, sharded inputs)."""
    import concourse.bacc as bacc
    import concourse.mybir as mybir
    from concourse.tile import TileContext

    DT = getattr(mybir.dt, dt_mm)
    F32 = mybir.dt.float32
    nhalf = min(NHALF, bl)
    nh = (bl + nhalf - 1) // nhalf

    nc = bacc.Bacc(None, target_bir_lowering=False, debug=False)
    xT = nc.declare_dram_parameter("xT", [L, bl], DT, isOutput=False)
    wmain_d = nc.declare_dram_parameter("wmain", [44, 105], DT, isOutput=False)
    wscan1_d = nc.declare_dram_parameter("wscan1", [126, 126], DT, isOutput=False)
    wqv_d = nc.declare_dram_parameter("wqv", [G, C, 126], DT, isOutput=False)
    winit_d = nc.declare_dram_parameter("winit", [7, 126], DT, isOutput=False)
    out_d = nc.declare_dram_parameter("out", [3 * L, bl], F32, isOutput=True)

    with TileContext(nc) as tc:
        with (
            tc.tile_pool(name="consts", bufs=1) as consts,
            tc.tile_pool(name="xpool", bufs=2 * G) as xpool,
            tc.tile_pool(name="spool", bufs=3) as spool,
            tc.tile_pool(name="ypool", bufs=6) as ypool,
            tc.tile_pool(name="ypsum", bufs=4, space="PSUM") as ypsum,
            tc.tile_pool(name="spsum", bufs=2, space="PSUM") as spsum,
        ):
            wmain = consts.tile([44, 105], DT)
            nc.sync.dma_start(out=wmain[:], in_=wmain_d[:])
            wscan1 = consts.tile([126, 126], DT)
            nc.sync.dma_start(out=wscan1[:], in_=wscan1_d[:])
            wqv = consts.tile([C, G * 126], DT)
            for i in range(G):
                nc.sync.dma_start(out=wqv[:, i * 126:(i + 1) * 126], in_=wqv_d[i])
            winit = consts.tile([7, 126], DT)
            nc.sync.dma_start(out=winit[:], in_=winit_d[:])
            xinit = consts.tile([7, bl], DT)
            nc.sync.dma_start(out=xinit[:], in_=xT[0:7, :])

            # --- init: y_0 rows and s_0 state (zeros elsewhere by construction)
            ip = spsum.tile([126, bl], F32, tag="sp")
            for h in range(nh):
                hs = slice(h * nhalf, (h + 1) * nhalf)
                nc.tensor.matmul(ip[:, hs], lhsT=winit[:], rhs=xinit[:, hs],
                                 start=True, stop=True)
            sprev = spool.tile([126, bl], DT, tag="sprev")
            nc.vector.tensor_copy(out=sprev[:], in_=ip[:])
            y0 = ypool.tile([3, bl], F32)
            nc.scalar.copy(out=y0[:], in_=ip[0:3, :])
            nc.sync.dma_start(out=out_d[0:3, :], in_=y0[:])

            for g_ in range(NG):
                xg = []
                for i in range(G):
                    t0 = 1 + C * (G * g_ + i)
                    xt = xpool.tile([44, bl], DT, tag="xg")
                    nc.sync.dma_start(out=xt[0:C, :], in_=xT[t0:t0 + C, :])
                    xg.append(xt)

                # --- group scan: all 13 chunk-entry states + next group state
                sp = spsum.tile([126, bl], F32, tag="sp")
                for h in range(nh):
                    hs = slice(h * nhalf, (h + 1) * nhalf)
                    nc.tensor.matmul(sp[:, hs], lhsT=wscan1[:], rhs=sprev[:, hs],
                                     start=True, stop=False)
                    for i in range(G):
                        nc.tensor.matmul(sp[:, hs],
                                         lhsT=wqv[:, i * 126:(i + 1) * 126],
                                         rhs=xg[i][0:C, hs],
                                         start=False, stop=(i == G - 1))
                sprev_g = spool.tile([126, bl], DT, tag="sprev")
                nc.vector.tensor_copy(out=sprev_g[:], in_=sp[:])
                # scatter sigma_i into rows 35..43 of each chunk tile
                for i in range(G):
                    nc.sync.dma_start(out=xg[i][C:C + 9, :],
                                      in_=sprev_g[9 * i:9 * i + 9, :])

                # --- pass 2: per-chunk outputs
                for i in range(G):
                    r0 = 3 * (1 + C * (G * g_ + i))
                    for h in range(nh):
                        hs = slice(h * nhalf, (h + 1) * nhalf)
                        yp = ypsum.tile([105, nhalf], F32, tag="yp")
                        nc.tensor.matmul(yp[:], lhsT=wmain[:], rhs=xg[i][:, hs],
                                         start=True, stop=True)
                        ysb = ypool.tile([105, nhalf], F32, tag="ysb")
                        if h % 2 == 0:
                            nc.vector.tensor_copy(out=ysb[:], in_=yp[:])
                        else:
                            nc.scalar.copy(out=ysb[:], in_=yp[:])
                        nc.sync.dma_start(out=out_d[r0:r0 + 105, hs], in_=ysb[:])
                sprev = sprev_g
    nc.compile()
    return nc


def _prep_inputs(x, alpha, beta, gamma, dt_mm=DT_MM):
    xs = np.asarray(x, dtype=np.float32).reshape(B, L)
    coeffs = _build_coeffs(float(alpha), float(beta), float(gamma))
    in_maps = []
    for m in range(NCORES):
        xT_m = np.ascontiguousarray(xs[m * BL:(m + 1) * BL].T)  # (L, BL)
        in_maps.append({"xT": xT_m, **coeffs})
    return in_maps


LAST_RESULT = None  # BassKernelResults of the most recent kernel() call


def kernel(x, alpha, beta, gamma):
    global LAST_RESULT
    from concourse.bass_utils import run_bass_kernel_spmd

    nc = build_bass(BL, DT_MM)
    in_maps = _prep_inputs(x, alpha, beta, gamma, DT_MM)
    res = run_bass_kernel_spmd(nc, in_maps, core_ids=list(range(NCORES)))
    LAST_RESULT = res
    outs = [r["out"] for r in res.results]          # each (3L, BL) float32
    y = np.empty((B, L, 3), np.float32)
    for m in range(NCORES):
        y[m * BL:(m + 1) * BL] = outs[m].T.reshape(BL, L, 3)
    return y


# revision 8
# speedup vs baseline: 1.6653x; 1.6653x over previous
"""Holt-Winters exponential smoothing (level/trend/seasonal, P=7) on 8 Trainium2
NeuronCores.

Math: the per-row recurrence is linear in a 9-dim state
s = [level, trend, buf_0..buf_6]:  s_t = A_{t%7} s_{t-1} + c_{t%7} x_t.
Steps t=1..4095 are processed in 117 chunks of C=35 steps (35 % 7 == 0 so every
chunk sees the same slot pattern and shares one coefficient set), grouped into
9 groups of G=13 chunks.  Per chunk the outputs are a matmul
  Y_c (105,B) = [Wm | U]^T @ [X_c (35,B); sigma_c (9,B)]
and the chunk-entry states sigma_c come from a per-group prefix-scan matmul
over the group's stacked inputs.  All heavy compute runs on the TensorEngine;
the only sequential dependency is the 9-link group chain.

Sharding: pure data-parallel over the batch axis (1024 rows per core).
"""

import numpy as np

P = 7
C = 35            # chunk size (steps); 35 % 7 == 0
G = 13            # chunks per group
NG = 9            # groups; NG*G*C == L-1
L = 4096
B = 8192
NCORES = 8
BL = B // NCORES  # 1024 batch rows per core
NHALF = 512       # matmul moving-dim tile (fp32 PSUM bank limit)

DT_MM = "float32r"  # matmul operand dtype: "float32" (exact) or "float32r" (fast)


def _sigmoid(z):
    return 1.0 / (1.0 + np.exp(-z))


def _step_mats(a, b, g):
    """A_i (9x9), c_i (9,) for seasonal slot i, float64."""
    A, c = [], []
    for i in range(P):
        col = 2 + i
        Ai = np.zeros((9, 9), np.float64)
        ci = np.zeros(9, np.float64)
        Ai[0, 0] = 1 - a
        Ai[0, 1] = 1 - a
        Ai[0, col] += -a
        Ai[1, 0] = -a * b
        Ai[1, 1] = 1 - a * b
        Ai[1, col] += -a * b
        for j in range(P):
            Ai[2 + j, 2 + j] = 1.0
        Ai[col, :] = 0.0
        Ai[col, 0] = -g * (1 - a)
        Ai[col, 1] = -g * (1 - a)
        Ai[col, col] = g * a + 1 - g
        ci[0] = a
        ci[1] = a * b
        ci[col] = g * (1 - a)
        A.append(Ai)
        c.append(ci)
    return A, c


def _build_coeffs(alpha, beta, gamma):
    """Host-precomputed stationary matrices (float64 -> float32).

    Returns dict of lhsT-layout arrays:
      wmain  (44, 105): pass-2; K rows 0..34 = X coeffs, 35..43 = sigma coeffs
      wscan1 (126, 126): scan mm over previous group's state tile (rows 117..125)
      wqv    (13, 35, 126): scan mm K-tile for each chunk position in the group
      winit  (7, 126): init mm: out rows 0..2 = y_0, rows 117..125 = s_0
    """
    a, b, g = _sigmoid(alpha), _sigmoid(beta), _sigmoid(gamma)
    A, c = _step_mats(a, b, g)
    slots = [(1 + k) % P for k in range(C)]

    Phi = np.zeros((C, 9, 9), np.float64)
    w = np.zeros((C, C, 9), np.float64)
    cur = np.eye(9)
    for k in range(C):
        i = slots[k]
        if k > 0:
            w[k, :k] = w[k - 1, :k] @ A[i].T
        w[k, k] = c[i]
        cur = A[i] @ cur
        Phi[k] = cur
    T = Phi[C - 1]
    V = w[C - 1].T.copy()  # (9, C)

    wmain = np.zeros((44, 105), np.float64)
    for k in range(C):
        sel = [0, 1, 2 + slots[k]]
        wmain[35:44, 3 * k:3 * k + 3] = Phi[k][sel].T          # U part
        for j in range(k + 1):
            wmain[j, 3 * k:3 * k + 3] = w[k, j][sel]            # Wm part

    Tpow = [np.eye(9)]
    for _ in range(G + 1):
        Tpow.append(T @ Tpow[-1])

    wscan1 = np.zeros((126, 126), np.float64)
    for j in range(G + 1):
        wscan1[117:126, 9 * j:9 * j + 9] = Tpow[j].T
    wqv = np.zeros((G, C, 126), np.float64)
    for i in range(G):
        for j in range(i + 1, G + 1):
            wqv[i, :, 9 * j:9 * j + 9] = (Tpow[j - 1 - i] @ V).T

    winit = np.zeros((7, 126), np.float64)
    winit[0, 0] = 1.0
    winit[0, 1] = -1.0
    winit[1, 1] = 1.0
    winit[0, 117] = 1.0
    winit[0, 118] = -1.0
    winit[1, 118] = 1.0
    for j in range(P):
        winit[j, 119 + j] += 1.0
        winit[0, 119 + j] += -1.0

    return {k: v.astype(np.float32) for k, v in
            dict(wmain=wmain, wscan1=wscan1, wqv=wqv, winit=winit).items()}


def build_bass(bl=BL, dt_mm=DT_MM):
    """Build the per-core Bass module (SPMD: same module, sharded inputs)."""
    import concourse.bacc as bacc
    import concourse.mybir as mybir
    from concourse.tile import TileContext

    DT = getattr(mybir.dt, dt_mm)
    F32 = mybir.dt.float32
    nhalf = min(NHALF, bl)
    nh = (bl + nhalf - 1) // nhalf

    nc = bacc.Bacc(None, target_bir_lowering=False, debug=False)
    xT = nc.declare_dram_parameter("xT", [L, bl], DT, isOutput=False)
    wmain_d = nc.declare_dram_parameter("wmain", [44, 105], DT, isOutput=False)
    wscan1_d = nc.declare_dram_parameter("wscan1", [126, 126], DT, isOutput=False)
    wqv_d = nc.declare_dram_parameter("wqv", [G, C, 126], DT, isOutput=False)
    winit_d = nc.declare_dram_parameter("winit", [7, 126], DT, isOutput=False)
    out_d = nc.declare_dram_parameter("out", [3 * L, bl], F32, isOutput=True)

    with TileContext(nc) as tc:
        with (
            tc.tile_pool(name="consts", bufs=1) as consts,
            tc.tile_pool(name="xpool", bufs=2 * G) as xpool,
            tc.tile_pool(name="spool", bufs=3) as spool,
            tc.tile_pool(name="ypool", bufs=6) as ypool,
            tc.tile_pool(name="ypsum", bufs=4, space="PSUM") as ypsum,
            tc.tile_pool(name="spsum", bufs=2, space="PSUM") as spsum,
        ):
            wmain = consts.tile([44, 105], DT)
            nc.sync.dma_start(out=wmain[:], in_=wmain_d[:])
            wscan1 = consts.tile([126, 126], DT)
            nc.sync.dma_start(out=wscan1[:], in_=wscan1_d[:])
            wqv = consts.tile([C, G * 126], DT)
            for i in range(G):
                nc.sync.dma_start(out=wqv[:, i * 126:(i + 1) * 126], in_=wqv_d[i])
            winit = consts.tile([7, 126], DT)
            nc.sync.dma_start(out=winit[:], in_=winit_d[:])
            xinit = consts.tile([7, bl], DT)
            nc.sync.dma_start(out=xinit[:], in_=xT[0:7, :])

            # --- init: y_0 rows and s_0 state (zeros elsewhere by construction)
            ip = spsum.tile([126, bl], F32, tag="sp")
            for h in range(nh):
                hs = slice(h * nhalf, (h + 1) * nhalf)
                nc.tensor.matmul(ip[:, hs], lhsT=winit[:], rhs=xinit[:, hs],
                                 start=True, stop=True)
            sprev = spool.tile([126, bl], DT, tag="sprev")
            nc.vector.tensor_copy(out=sprev[:], in_=ip[:])
            y0 = ypool.tile([3, bl], F32)
            nc.scalar.copy(out=y0[:], in_=ip[0:3, :])
            nc.sync.dma_start(out=out_d[0:3, :], in_=y0[:])

            for g_ in range(NG):
                xg = []
                for i in range(G):
                    t0 = 1 + C * (G * g_ + i)
                    xt = xpool.tile([44, bl], DT, tag="xg")
                    nc.sync.dma_start(out=xt[0:C, :], in_=xT[t0:t0 + C, :])
                    xg.append(xt)

                # --- group scan: all 13 chunk-entry states + next group state
                sp = spsum.tile([126, bl], F32, tag="sp")
                for h in range(nh):
                    hs = slice(h * nhalf, (h + 1) * nhalf)
                    nc.tensor.matmul(sp[:, hs], lhsT=wscan1[:], rhs=sprev[:, hs],
                                     start=True, stop=False)
                    for i in range(G):
                        nc.tensor.matmul(sp[:, hs],
                                         lhsT=wqv[:, i * 126:(i + 1) * 126],
                                         rhs=xg[i][0:C, hs],
                                         start=False, stop=(i == G - 1))
                sprev_g = spool.tile([126, bl], DT, tag="sprev")
                nc.vector.tensor_copy(out=sprev_g[:], in_=sp[:])
                # scatter sigma_i into rows 35..43 of each chunk tile
                for i in range(G):
                    nc.sync.dma_start(out=xg[i][C:C + 9, :],
                                      in_=sprev_g[9 * i:9 * i + 9, :])

                # --- pass 2: per-chunk outputs
                for i in range(G):
                    r0 = 3 * (1 + C * (G * g_ + i))
                    for h in range(nh):
                        hs = slice(h * nhalf, (h + 1) * nhalf)
                        yp = ypsum.tile([105, nhalf], F32, tag="yp")
                        nc.tensor.matmul(yp[:], lhsT=wmain[:], rhs=xg[i][:, hs],
                                         start=True, stop=True)
                        ysb = ypool.tile([105, nhalf], F32, tag="ysb")
                        if h % 2 == 0:
                            nc.vector.tensor_copy(out=ysb[:], in_=yp[:])
                        else:
                            nc.scalar.copy(out=ysb[:], in_=yp[:])
                        nc.sync.dma_start(out=out_d[r0:r0 + 105, hs], in_=ysb[:])
                sprev = sprev_g
    nc.compile()
    return nc


def _prep_inputs(x, alpha, beta, gamma, dt_mm=DT_MM):
    xs = np.asarray(x, dtype=np.float32).reshape(B, L)
    coeffs = _build_coeffs(float(alpha), float(beta), float(gamma))
    in_maps = []
    for m in range(NCORES):
        xT_m = np.ascontiguousarray(xs[m * BL:(m + 1) * BL].T)  # (L, BL)
        in_maps.append({"xT": xT_m, **coeffs})
    return in_maps


LAST_RESULT = None  # BassKernelResults of the most recent kernel() call


def kernel(x, alpha, beta, gamma):
    global LAST_RESULT
    from concourse.bass_utils import run_bass_kernel_spmd

    nc = build_bass(BL, DT_MM)
    in_maps = _prep_inputs(x, alpha, beta, gamma, DT_MM)
    res = run_bass_kernel_spmd(nc, in_maps, core_ids=list(range(NCORES)))
    LAST_RESULT = res
    outs = [r["out"] for r in res.results]          # each (3L, BL) float32
    y = np.empty((B, L, 3), np.float32)
    for m in range(NCORES):
        y[m * BL:(m + 1) * BL] = outs[m].T.reshape(BL, L, 3)
    return y
